# revision 1
# baseline (speedup 1.0000x reference)
"""AdaMoLE (LoRA-MoE routing) Trainium2 kernel, data-parallel over tokens on 8 cores.

Math (per token n):
    logits = x @ Wr.T + br                 [E]
    gate   = softmax(logits)
    thr    = sigmoid(x @ Wt.T + bt) / E    [1]
    w      = relu(gate - thr); w /= max(sum(w), eps)
    h      = x @ A_all                     [E*R]   (A_all = concat_e A_e)
    out    = (h * rep(w) * SCALING) @ B_all        (B_all = concat_e B_e)

Each core takes 2048 tokens. x is passed host-side in a transposed,
chunk-major layout so the contraction dim (d) lands on SBUF partitions and
every DMA is contiguous; f32 -> bf16 cast happens inline in the SWDGE DMA.
Cross-expert (partition-dim) softmax reductions are done with tiny PE
ones-matmuls; weight broadcast across the er=128 lanes with a replication
matmul. Output written f32.
"""

import sys

sys.path.insert(0, "/opt/trn_rl_repo")

import numpy as np
import ml_dtypes

import concourse.bacc as bacc
import concourse.mybir as mybir
import concourse.tile as tile
from concourse.bass_utils import run_bass_kernel_spmd
from contextlib import ExitStack

F32 = mybir.dt.float32
BF16 = mybir.dt.bfloat16
AF = mybir.ActivationFunctionType

B, S, D, DOUT = 4, 4096, 4096, 4096
R, E = 16, 8
SCALING = 8.0 / R  # lora_alpha / r
NCORES = 8
N = B * S
NTOK = N // NCORES        # 2048 tokens per core
# token blocks per core; fine 128-token blocks minimize pipeline fill/drain
# (HW A/B: finer blocking beat coarse 512-token blocks consistently)
BLOCKS = [128] * 16
NBLK = len(BLOCKS)
TBLK = 512                # max block size (psum/sbuf tile sizing)
NDC = D // 128            # 32 contraction chunks
ER = E * R                # 128
NOB = DOUT // 512         # 8 output column blocks

_CACHE = {}


def _build(reps=1, loop=False):
    nc = bacc.Bacc("TRN2", debug=False, num_devices=NCORES)

    X = nc.declare_dram_parameter("X", [128, NDC * NTOK], F32, isOutput=False)
    Aw = nc.declare_dram_parameter("Aw", [128, NDC * ER], BF16, isOutput=False)
    Wc = nc.declare_dram_parameter("Wc", [128, NDC * 9], BF16, isOutput=False)
    Bl = nc.declare_dram_parameter("Bl", [ER, DOUT], BF16, isOutput=False)
    REPs = nc.declare_dram_parameter("REPs", [E, ER], F32, isOutput=False)
    ONES8 = nc.declare_dram_parameter("ONES8", [E, 1], F32, isOutput=False)
    SEL9 = nc.declare_dram_parameter("SEL9", [9, E], F32, isOutput=False)
    BC1 = nc.declare_dram_parameter("BC1", [1, E], F32, isOutput=False)
    BR = nc.declare_dram_parameter("BR", [E, 1], F32, isOutput=False)
    BT8 = nc.declare_dram_parameter("BT8", [E, 1], F32, isOutput=False)
    OUT = nc.declare_dram_parameter("out", [NTOK, DOUT], F32, isOutput=True)

    with tile.TileContext(nc) as tc, ExitStack() as ctx:
        wpool = ctx.enter_context(tc.tile_pool(name="w", bufs=1))
        xpool = ctx.enter_context(tc.tile_pool(name="x", bufs=6))
        opool = ctx.enter_context(tc.tile_pool(name="o", bufs=4))
        spool = ctx.enter_context(tc.tile_pool(name="s", bufs=2))
        hwpool = ctx.enter_context(tc.tile_pool(name="hw", bufs=2))
        ph = ctx.enter_context(tc.tile_pool(name="ph", bufs=2, space="PSUM"))
        pr = ctx.enter_context(tc.tile_pool(name="pr", bufs=1, space="PSUM"))
        pm = ctx.enter_context(tc.tile_pool(name="pm", bufs=2, space="PSUM"))
        po = ctx.enter_context(tc.tile_pool(name="po", bufs=3, space="PSUM"))

        A_sb = wpool.tile([128, NDC * ER], BF16, tag="A")
        nc.sync.dma_start(out=A_sb[:], in_=Aw[:])
        Wc_sb = wpool.tile([128, NDC * 9], BF16, tag="Wc")
        nc.sync.dma_start(out=Wc_sb[:], in_=Wc[:])
        REP_sb = wpool.tile([E, ER], F32, tag="REP")
        nc.sync.dma_start(out=REP_sb[:], in_=REPs[:])
        ONES_sb = wpool.tile([E, 1], F32, tag="ONES")
        nc.sync.dma_start(out=ONES_sb[:], in_=ONES8[:])
        SEL9_sb = wpool.tile([9, E], F32, tag="SEL9")
        nc.sync.dma_start(out=SEL9_sb[:], in_=SEL9[:])
        BC1_sb = wpool.tile([1, E], F32, tag="BC1")
        nc.sync.dma_start(out=BC1_sb[:], in_=BC1[:])
        BR_sb = wpool.tile([E, 1], F32, tag="BR")
        nc.sync.dma_start(out=BR_sb[:], in_=BR[:])
        BT8_sb = wpool.tile([E, 1], F32, tag="BT8")
        nc.sync.dma_start(out=BT8_sb[:], in_=BT8[:])
        # B is not needed until the first mm2 (~25us in); load it after the
        # small consts so they don't queue behind its 1MB on the sync ring
        B_sb = wpool.tile([ER, DOUT], BF16, tag="B")
        nc.sync.dma_start(out=B_sb[:], in_=Bl[:])

        def emit_block(t0, bs):
            x0 = NDC * t0
            ncols = NDC * bs
            xb = xpool.tile([128, ncols], BF16, tag="xb")
            # f32 -> bf16 cast inline in SWDGE DMA; 1 MB (f32 side) sub-DMAs
            for c0 in range(0, ncols, 2048):
                c1 = min(c0 + 2048, ncols)
                nc.gpsimd.dma_start(out=xb[:, c0:c1], in_=X[:, x0 + c0 : x0 + c1])

            # routing logits first so the routing chain (PE/DVE/ACT ping-pong)
            # overlaps the h accumulation
            r_ps = pr.tile([9, bs], F32, tag="r")
            for dc in range(NDC):
                nc.tensor.matmul(
                    r_ps[:],
                    Wc_sb[:, dc * 9 : (dc + 1) * 9],
                    xb[:, dc * bs : (dc + 1) * bs],
                    start=(dc == 0),
                    stop=(dc == NDC - 1),
                )
            # h[er, t] accumulated over the 32 d-chunks
            h_ps = ph.tile([ER, bs], F32, tag="h")
            for dc in range(NDC):
                nc.tensor.matmul(
                    h_ps[:],
                    A_sb[:, dc * ER : (dc + 1) * ER],
                    xb[:, dc * bs : (dc + 1) * bs],
                    start=(dc == 0),
                    stop=(dc == NDC - 1),
                )

            # ---- routing math (all partition-slices start at base 0) ----
            r_sb = spool.tile([9, bs], F32, tag="rsb")
            nc.scalar.activation(r_sb[:], r_ps[:], AF.Copy)
            eexp = spool.tile([E, bs], F32, tag="eexp")
            nc.scalar.activation(eexp[:], r_ps[0:E, :], AF.Exp, bias=BR_sb[:])
            S1 = pm.tile([1, bs], F32, tag="pm")
            nc.tensor.matmul(S1[:], ONES_sb[:], eexp[:], start=True, stop=True)
            sg = spool.tile([1, bs], F32, tag="sg")
            nc.vector.reciprocal(sg[:], S1[:])
            GD8 = pm.tile([E, bs], F32, tag="pm")
            nc.tensor.matmul(GD8[:], BC1_sb[:], sg[:], start=True, stop=True)
            t1 = spool.tile([E, bs], F32, tag="t1")
            nc.vector.tensor_mul(t1[:], eexp[:], GD8[:])  # gate
            RT8 = pm.tile([E, bs], F32, tag="pm")
            nc.tensor.matmul(RT8[:], SEL9_sb[:], r_sb[:], start=True, stop=True)
            th8 = spool.tile([E, bs], F32, tag="th8")
            nc.scalar.activation(th8[:], RT8[:], AF.Sigmoid, bias=BT8_sb[:])
            # adapted = gate - sigmoid(rt)/E  =  (th8 * -1/E) + t1
            adapted = spool.tile([E, bs], F32, tag="ad")
            nc.vector.scalar_tensor_tensor(
                adapted[:], th8[:], -1.0 / E, t1[:],
                mybir.AluOpType.mult, mybir.AluOpType.add,
            )
            wrelu = spool.tile([E, bs], F32, tag="wr")
            nc.vector.tensor_relu(wrelu[:], adapted[:])
            S2 = pm.tile([1, bs], F32, tag="pm")
            nc.tensor.matmul(S2[:], ONES_sb[:], wrelu[:], start=True, stop=True)
            smax = spool.tile([1, bs], F32, tag="sm")
            nc.vector.tensor_scalar_max(smax[:], S2[:], 1e-30)
            srecip = spool.tile([1, bs], F32, tag="sr")
            nc.vector.reciprocal(srecip[:], smax[:])
            SR8 = pm.tile([E, bs], F32, tag="pm")
            nc.tensor.matmul(SR8[:], BC1_sb[:], srecip[:], start=True, stop=True)
            wn = spool.tile([E, bs], F32, tag="wn")
            nc.vector.tensor_mul(wn[:], wrelu[:], SR8[:])
            WREPp = pm.tile([ER, bs], F32, tag="pm")
            nc.tensor.matmul(WREPp[:], REP_sb[:], wn[:], start=True, stop=True)
            WREP = spool.tile([ER, bs], F32, tag="WREP")
            nc.scalar.activation(WREP[:], WREPp[:], AF.Copy)
            hw = hwpool.tile([ER, bs], BF16, tag="hw")
            nc.vector.tensor_mul(hw[:], WREP[:], h_ps[:])

            # ---- second matmul + output ----
            # (HW A/B x3: spreading stores across the two HWDGE rings makes
            # no measurable difference on this device; keep the sync ring)
            for t4 in range(bs // 128):
                row = t0 + t4 * 128
                o_sb = opool.tile([128, DOUT], F32, tag="osb")
                for nb in range(NOB):
                    o_ps = po.tile([128, 512], F32, tag="o")
                    nc.tensor.matmul(
                        o_ps[:],
                        hw[:, t4 * 128 : (t4 + 1) * 128],
                        B_sb[:, nb * 512 : (nb + 1) * 512],
                        start=True,
                        stop=True,
                    )
                    if nb % 2 == 0:
                        nc.scalar.activation(
                            o_sb[:, nb * 512 : (nb + 1) * 512], o_ps[:], AF.Copy
                        )
                    else:
                        nc.vector.tensor_copy(
                            o_sb[:, nb * 512 : (nb + 1) * 512], o_ps[:]
                        )
                nc.sync.dma_start(out=OUT[row : row + 128, :], in_=o_sb[:])

        def emit_all():
            t0 = 0
            for bs in BLOCKS:
                emit_block(t0, bs)
                t0 += bs

        if loop:
            with tc.For_i(0, reps, 1):
                emit_all()
        else:
            for r in range(reps):
                emit_all()

    nc.compile()
    return nc


def _prep_consts(Wr, br, Wt, bt, A, Bw):
    bf = ml_dtypes.bfloat16
    A_all = np.ascontiguousarray(
        np.asarray(A, np.float32).transpose(1, 0, 2).reshape(D, ER)
    )  # [d, er]
    A_host = np.ascontiguousarray(
        A_all.reshape(NDC, 128, ER).transpose(1, 0, 2).reshape(128, NDC * ER)
    ).astype(bf)
    Wcat = np.concatenate(
        [np.asarray(Wr, np.float32).T, np.asarray(Wt, np.float32).T], axis=1
    )  # [d, 9]
    Wc_host = np.ascontiguousarray(
        Wcat.reshape(NDC, 128, 9).transpose(1, 0, 2).reshape(128, NDC * 9)
    ).astype(bf)
    B_host = np.ascontiguousarray(np.asarray(Bw, np.float32).reshape(ER, DOUT)).astype(bf)
    REPh = np.zeros((E, ER), np.float32)
    for e in range(E):
        REPh[e, e * R : (e + 1) * R] = SCALING
    ONESh = np.ones((E, 1), np.float32)
    SEL9h = np.zeros((9, E), np.float32)
    SEL9h[8, :] = 1.0
    BC1h = np.ones((1, E), np.float32)
    BRh = np.asarray(br, np.float32).reshape(E, 1)
    BT8h = np.full((E, 1), np.float32(np.asarray(bt).reshape(())), np.float32)
    return {
        "Aw": A_host,
        "Wc": Wc_host,
        "Bl": B_host,
        "REPs": REPh,
        "ONES8": ONESh,
        "SEL9": SEL9h,
        "BC1": BC1h,
        "BR": BRh,
        "BT8": BT8h,
    }


def _prep_x(xs):
    """Per-core shard [NTOK, D] -> [128, sum(NDC*bs)] with per-block
    [p, dc, t] layout so every DMA slice is contiguous."""
    parts = []
    t0 = 0
    for bs in BLOCKS:
        blkarr = (
            xs[t0 : t0 + bs].reshape(bs, NDC, 128).transpose(2, 1, 0).reshape(128, NDC * bs)
        )
        parts.append(blkarr)
        t0 += bs
    return np.ascontiguousarray(np.concatenate(parts, axis=1))


def kernel(x, Wr, br, Wt, bt, A, Bw, _trace=False, _trace_kwargs=None):
    if "nc" not in _CACHE:
        _CACHE["nc"] = _build()
    nc = _CACHE["nc"]

    consts = _prep_consts(Wr, br, Wt, bt, A, Bw)
    xf = np.asarray(x, np.float32).reshape(N, D)
    in_maps = []
    for c in range(NCORES):
        Xh = _prep_x(xf[c * NTOK : (c + 1) * NTOK])
        in_maps.append({"X": Xh, **consts})

    res = run_bass_kernel_spmd(
        nc,
        in_maps,
        core_ids=list(range(NCORES)),
        trace=_trace,
        **(_trace_kwargs or {}),
    )
    out = np.concatenate([res.results[c]["out"] for c in range(NCORES)], axis=0)
    if _trace:
        _CACHE["last_res"] = res
    return out.reshape(B, S, DOUT).astype(np.float32)

